# revision 17
# baseline (speedup 1.0000x reference)
import sys

sys.path.insert(0, "/opt/trn_rl_repo")

import os

import numpy as np
import ml_dtypes

import concourse.bass as bass
import concourse.mybir as mybir
from concourse.tile import TileContext
from concourse.bass_utils import run_bass_kernel_spmd


def _split_multiwait_drains(nc):
    """This walrus build only encodes one sem-wait per instruction; hoist
    extra waits onto preceding same-engine NoOps (engines execute their
    instructions in block order, so the waits remain equivalent)."""
    import bass_rust

    uid = [0]
    for fn in nc.m.functions:
        for blk in fn.blocks:
            out, changed = [], False
            for inst in blk.instructions:
                si = getattr(inst, "sync_info", None)
                if si is not None and si.on_wait and len(si.on_wait) > 1:
                    waits = list(si.on_wait)
                    for w in waits[:-1]:
                        n = bass_rust.InstNoOp(name=f"syncw_{uid[0]}", ins=[], outs=[])
                        uid[0] += 1
                        n.engine = inst.engine
                        n.sync_info = bass_rust.SyncInfo(on_wait=[w], on_update=[])
                        out.append(n)
                    si.on_wait = [waits[-1]]
                    changed = True
                out.append(inst)
            if changed:
                blk.instructions = out


B, C, H, W = 4, 128, 128, 128
HEADS, DH = 8, 16
WL = 64  # per-core w-slice (2 cores per batch image)
N_CORES = 8
MB = 16  # sequences per megabatch

FP32 = mybir.dt.float32
BF16 = mybir.dt.bfloat16
BF16_NP = ml_dtypes.bfloat16

AXES = ("h", "w")


def _build_nc():
    nc = bass.Bass()

    x = nc.declare_dram_parameter("x", [C, H * W], FP32, isOutput=False)
    phw = nc.declare_dram_parameter("phw", [C, H * W], BF16, isOutput=False)
    sel = nc.declare_dram_parameter("sel", [C, C], BF16, isOutput=False)
    bosum = nc.declare_dram_parameter("bosum", [C, 1], FP32, isOutput=False)
    wts = {}
    for ax in AXES:
        for wn in ("wk", "woa", "wob") + tuple(f"wqm{h}" for h in range(8)):
            wts[f"{wn}_{ax}"] = nc.declare_dram_parameter(
                f"{wn}_{ax}", [C, C], BF16, isOutput=False
            )
        wts[f"wv_{ax}"] = nc.declare_dram_parameter(
            f"wv_{ax}", [C, 2 * C], BF16, isOutput=False
        )
    out = nc.declare_dram_parameter("out", [C, H * WL], FP32, isOutput=True)

    EXP = mybir.ActivationFunctionType.Exp
    LOG = mybir.ActivationFunctionType.Ln
    ADD = mybir.AluOpType.add
    MULT = mybir.AluOpType.mult

    QCH = 512  # psum chunk (one bank of fp32)

    with TileContext(nc) as tc:
        with (
            tc.tile_pool(name="big", bufs=1) as big,
            tc.tile_pool(name="wpool", bufs=1) as wpool,
            tc.tile_pool(name="slab", bufs=1) as slab_pool,
            tc.tile_pool(name="etsb", bufs=3) as etsb_pool,
            tc.tile_pool(name="misc", bufs=2) as misc_pool,
            tc.tile_pool(name="xin", bufs=2) as xin_pool,
            tc.tile_pool(name="scps", bufs=1, space="PSUM") as scps_pool,
            tc.tile_pool(name="ups", bufs=1, space="PSUM") as ups_pool,
            tc.tile_pool(name="vps", bufs=2, space="PSUM") as vps_pool,
            tc.tile_pool(name="pps", bufs=2, space="PSUM") as pps_pool,
        ):
            # ---- stage weights (DMAs issued from sync engine; it is idle) ----
            wsb = {}
            for k, t in wts.items():
                wt = wpool.tile(list(t.shape), t.dtype, tag=k)
                nc.sync.dma_start(out=wt[:], in_=t[:])
                wsb[k] = wt
            sel_sb = wpool.tile([C, C], BF16, tag="sel")
            nc.sync.dma_start(out=sel_sb[:], in_=sel[:])
            bos_sb = wpool.tile([C, 1], FP32, tag="bos")
            nc.sync.dma_start(out=bos_sb[:], in_=bosum[:])

            # ---- resident slabs ----
            xp_sb = big.tile([C, H * W], BF16, tag="xp")
            th_sb = big.tile([C, H * WL], BF16, tag="th")  # [c, h, wl]
            tw_sb = big.tile([C, H * WL], BF16, tag="tw")  # [c, h, wl]

            # xp = x + phw (x streamed through small rotating tiles)
            NCH = 8
            CH = (H * W) // NCH
            for k in range(NCH):
                cs = slice(k * CH, (k + 1) * CH)
                xt = xin_pool.tile([C, CH], FP32, tag="xin")
                nc.sync.dma_start(out=xt[:], in_=x[:, cs])
                nc.sync.dma_start(out=xp_sb[:, cs], in_=phw[:, cs])
                nc.vector.tensor_tensor(
                    out=xp_sb[:, cs], in0=xp_sb[:, cs], in1=xt[:], op=ADD
                )

            # v tiles with persistent ones columns
            v_tiles = []
            for j in range(3):
                vt = wpool.tile([C, 2 * C], BF16, tag=f"vsb{j}")
                vt3 = vt[:].rearrange("p (h c) -> p h c", c=32)
                nc.vector.memset(vt3[:, :, 16:], 1.0)
                v_tiles.append(vt)

            xp3 = xp_sb[:].rearrange("p (h w) -> p h w", w=W)

            def _copy_v(dst, src):
                nc.vector.tensor_copy(dst, src)

            def _copy_g(dst, src):
                nc.gpsimd.tensor_copy(dst, src)

            def _copy_s(dst, src):
                nc.scalar.copy(dst, src)

            # PSUM-reading copies: only DVE and Act may touch PSUM
            copy_fns = [_copy_v, _copy_s]
            eng_i = [0]

            def next_eng():
                e = copy_fns[eng_i[0] % 2]
                eng_i[0] += 1
                return e

            # shared slabs, sized for the larger (H) axis
            xw_sl = slab_pool.tile([C, MB * W], BF16, tag="xw")
            q_sl = slab_pool.tile([C, MB * 8 * H], BF16, tag="q")
            k_sl = slab_pool.tile([C, MB * W], BF16, tag="k")
            u_sl = slab_pool.tile([C, MB * 2 * H], BF16, tag="u")
            rn_sl = slab_pool.tile([C, MB * 2 * H], FP32, tag="rn")
            on_sl = slab_pool.tile([C, MB * 2 * H], BF16, tag="on")

            def run_axis(ax, nseq, nq, out_slab):
                """ax: 'h' (seq=w-col, nq=H) or 'w' (seq=h-row, nq=WL)."""
                n_mb = nseq // MB
                q4 = q_sl[:, : MB * 8 * nq].rearrange(
                    "p (s g q) -> p s g q", g=8, q=nq
                )

                for mb in range(n_mb):
                    # ---------- stage seq-major contiguous xp copy ----------
                    if ax == "h":
                        # xw cols (wl, h) <- xp[c, h, w-slice] transposed
                        xw = xw_sl[:, : MB * nq]
                        nc.gpsimd.tensor_copy(
                            xw.rearrange("p (s q) -> p s q", q=nq),
                            xp3[:, :, mb * MB : (mb + 1) * MB].rearrange(
                                "p h w -> p w h"
                            ),
                        )
                        mk_flat = xw  # kv positions = same cols
                        m_kv = H
                    else:
                        # q cols (h-row, w<WL) contiguous-staged; kv rows direct
                        xw = xw_sl[:, : MB * nq]
                        nc.gpsimd.tensor_copy(
                            xw.rearrange("p (s q) -> p s q", q=nq),
                            xp3[:, mb * MB : (mb + 1) * MB, 0:WL],
                        )
                        mk_flat = xp_sb[
                            :, mb * MB * W : (mb + 1) * MB * W
                        ]  # contiguous (h, w)
                        m_kv = W

                    # k projection first (no xw dependency; hides the DMA)
                    spk = QCH // m_kv
                    nkch = MB // spk
                    kd3 = k_sl[:, : MB * m_kv].rearrange("p (s q) -> p s q", q=m_kv)
                    for ck in range(nkch):
                        pt = pps_pool.tile([C, QCH], FP32, tag="pp")
                        nc.tensor.matmul(
                            pt[:],
                            wsb[f"wk_{ax}"][:],
                            mk_flat[:, ck * QCH : (ck + 1) * QCH],
                        )
                        next_eng()(
                            kd3[:, ck * spk : (ck + 1) * spk, :],
                            pt[:].rearrange("p (s q) -> p s q", q=m_kv),
                        )
                    # q projections: 8 masked weights; psum-chunked
                    spq = QCH // nq  # seqs per chunk
                    nqch = MB // spq
                    for h in range(8):
                        for ck in range(nqch):
                            pt = pps_pool.tile([C, QCH], FP32, tag="pp")
                            nc.tensor.matmul(
                                pt[:],
                                wsb[f"wqm{h}_{ax}"][:],
                                xw[:, ck * QCH : (ck + 1) * QCH],
                            )
                            next_eng()(
                                q4[:, ck * spq : (ck + 1) * spq, h, :],
                                pt[:].rearrange("p (s q) -> p s q", q=nq),
                            )

                    us4 = u_sl[:, : MB * 2 * nq].rearrange(
                        "p (v s q) -> p v s q", v=2, q=nq
                    )

                    # ---------- per-sequence attention ----------
                    for si in range(MB):
                        seq = mb * MB + si
                        if ax == "h":
                            rhs_kv = xw[:, si * H : (si + 1) * H]  # contiguous
                        else:
                            rhs_kv = xp3[:, seq, :]  # [C, W] contiguous

                        # v projection: stationary xp-seq, moving wv (aug)
                        v_ps = vps_pool.tile([C, 2 * C], FP32, tag="v")
                        nc.tensor.matmul(v_ps[:], rhs_kv, wsb[f"wv_{ax}"][:])
                        v_sb = v_tiles[si % 3]
                        vap_src = v_ps[:].rearrange("p (h c) -> p h c", c=32)[
                            :, :, :16
                        ]
                        vap_dst = v_sb[:].rearrange("p (h c) -> p h c", c=32)[
                            :, :, :16
                        ]
                        nc.scalar.copy(vap_dst, vap_src)

                        # scores: masked q blocks vs natural K stationary
                        # (moving capped at 512 cols by the matmul ISA)
                        et_sb = etsb_pool.tile([C, 8 * nq], BF16, tag="et")
                        sc_ps = scps_pool.tile([C, 8 * nq], FP32, tag="sc")
                        n_sc = max(1, (8 * nq) // 512)
                        scw = (8 * nq) // n_sc
                        for sck in range(n_sc):
                            nc.tensor.matmul(
                                sc_ps[:, sck * scw : (sck + 1) * scw],
                                k_sl[:, si * m_kv : (si + 1) * m_kv],
                                q_sl[
                                    :,
                                    si * 8 * nq + sck * scw : si * 8 * nq
                                    + (sck + 1) * scw,
                                ],
                            )
                        nc.scalar.activation(et_sb[:], sc_ps[:], EXP)

                        # AV: one matmul per half (4 heads at once + sum rows)
                        u_ps = ups_pool.tile([C, 8 * nq], FP32, tag="u")
                        nc.tensor.matmul(
                            u_ps[:, 0 : 4 * nq], v_sb[:, 0:C], et_sb[:, 0 : 4 * nq]
                        )
                        nc.tensor.matmul(
                            u_ps[:, 4 * nq : 8 * nq],
                            v_sb[:, C : 2 * C],
                            et_sb[:, 4 * nq : 8 * nq],
                        )
                        # compaction: per 32-row group g, gather block g of both
                        # halves -> u_sl[32g:32g+32, (si, :, :)]
                        up4 = u_ps[:].rearrange(
                            "p (v g q) -> p v g q", v=2, q=nq
                        )
                        for g in range(4):
                            next_eng()(
                                us4[32 * g : 32 * g + 32, :, si, :],
                                up4[32 * g : 32 * g + 32, :, g, :],
                            )

                    # ---------- batched normalize + output projection ----------
                    u_tot = MB * 2 * nq
                    nsch = u_tot // QCH
                    for ck in range(nsch):
                        cs = slice(ck * QCH, (ck + 1) * QCH)
                        rb_ps = pps_pool.tile([C, QCH], FP32, tag="pp")
                        nc.tensor.matmul(rb_ps[:], sel_sb[:], u_sl[:, cs])
                        ls_t = misc_pool.tile([C, QCH], FP32, tag="ls")
                        nc.scalar.activation(ls_t[:], rb_ps[:], LOG)
                        nc.scalar.activation(rn_sl[:, cs], ls_t[:], EXP, scale=-1.0)
                    for ck in range(2):
                        cs = slice(ck * u_tot // 2, (ck + 1) * u_tot // 2)
                        nc.gpsimd.tensor_tensor(
                            out=on_sl[:, cs], in0=u_sl[:, cs], in1=rn_sl[:, cs],
                            op=MULT,
                        )

                    spy = QCH // nq
                    nych = MB // spy
                    for ck in range(nych):
                        y_ps = pps_pool.tile([C, QCH], FP32, tag="pp")
                        nc.tensor.matmul(
                            y_ps[:],
                            wsb[f"woa_{ax}"][:],
                            on_sl[:, ck * QCH : (ck + 1) * QCH],
                            start=True, stop=False,
                        )
                        nc.tensor.matmul(
                            y_ps[:],
                            wsb[f"wob_{ax}"][:],
                            on_sl[:, u_tot // 2 + ck * QCH : u_tot // 2 + (ck + 1) * QCH],
                            start=False, stop=True,
                        )
                        if ax == "h":
                            # y cols (wl, h) -> th[c, h, wl]
                            dst = out_slab[:].rearrange("p (h w) -> p h w", w=WL)[
                                :, :,
                                mb * MB + ck * spy : mb * MB + (ck + 1) * spy,
                            ].rearrange("p h w -> p w h")
                            src = y_ps[:].rearrange("p (s q) -> p s q", q=nq)
                        else:
                            # y cols (h, w) -> tw[c, h, w] contiguous
                            dst = out_slab[
                                :,
                                (mb * MB + ck * spy) * nq : (mb * MB + (ck + 1) * spy) * nq,
                            ]
                            src = y_ps[:]
                        next_eng()(dst, src)

            run_axis("h", WL, H, th_sb)
            run_axis("w", H, WL, tw_sb)

            # ============ final: out = th + tw + x(local) + bias ============
            x3d = x[:].rearrange("p (h w) -> p h w", w=W)
            NFC = 8
            FH = (H * WL) // NFC  # flat cols per chunk
            HR = H // NFC  # h-rows per chunk
            for k in range(NFC):
                cs = slice(k * FH, (k + 1) * FH)
                xt = xin_pool.tile([C, FH], FP32, tag="xr")
                nc.sync.dma_start(
                    out=xt[:].rearrange("p (h w) -> p h w", w=WL),
                    in_=x3d[:, k * HR : (k + 1) * HR, 0:WL],
                )
                t1 = misc_pool.tile([C, FH], BF16, tag="f1")
                nc.gpsimd.tensor_tensor(
                    out=t1[:], in0=th_sb[:, cs], in1=tw_sb[:, cs], op=ADD
                )
                nc.gpsimd.tensor_scalar_add(out=t1[:], in0=t1[:], scalar1=bos_sb[:])
                ot = xin_pool.tile([C, FH], FP32, tag="ot")
                nc.vector.tensor_tensor(out=ot[:], in0=t1[:], in1=xt[:], op=ADD)
                nc.sync.dma_start(out=out[:, cs], in_=ot[:])

    _split_multiwait_drains(nc)
    return nc


_NC_CACHE = None


def _get_nc():
    global _NC_CACHE
    if _NC_CACHE is None:
        _NC_CACHE = _build_nc()
    return _NC_CACHE


def _host_prep(x, pos_h, pos_w, weights):
    """build per-core input maps"""
    scale = DH ** -0.5
    phw = (pos_h + pos_w)[0]  # [C, H, W]

    def grouped_rows(Wm, heads_sel):
        out = np.zeros((C, C), np.float32)
        for g, h in enumerate(heads_sel):
            out[32 * g : 32 * g + 16, :] = Wm[16 * h : 16 * h + 16, :]
        return out

    def v_aug(Wm):
        out = np.zeros((C, 2 * C), np.float32)
        for h in range(HEADS):
            out[:, 32 * h : 32 * h + 16] = Wm[:, 16 * h : 16 * h + 16]
        return out

    sel = np.zeros((C, C), np.float32)
    for q in range(C):
        sel[32 * (q // 32) + 16, q] = 1.0

    base = {"sel": sel.astype(BF16_NP)}
    for ax in AXES:
        Wq, Wk, Wv, Wo = weights[ax]
        Wqs = Wq * scale
        for h in range(HEADS):
            wm = np.zeros((C, C), np.float32)
            wm[:, 16 * h : 16 * h + 16] = Wqs[:, 16 * h : 16 * h + 16]
            base[f"wqm{h}_{ax}"] = wm.astype(BF16_NP)
        base[f"wk_{ax}"] = Wk.astype(BF16_NP)
        base[f"wv_{ax}"] = v_aug(Wv).astype(BF16_NP)
        base[f"woa_{ax}"] = grouped_rows(Wo, [0, 1, 2, 3]).astype(BF16_NP)
        base[f"wob_{ax}"] = grouped_rows(Wo, [4, 5, 6, 7]).astype(BF16_NP)

    in_maps = []
    for core in range(N_CORES):
        b, s = core // 2, core % 2
        xb = x[b]
        pb = phw
        if s == 1:
            xb = np.concatenate([xb[:, :, WL:], xb[:, :, :WL]], axis=2)
            pb = np.concatenate([pb[:, :, WL:], pb[:, :, :WL]], axis=2)
        m = dict(base)
        m["x"] = np.ascontiguousarray(xb.reshape(C, H * W), np.float32)
        m["phw"] = np.ascontiguousarray(pb.reshape(C, H * W)).astype(BF16_NP)
        in_maps.append(m)
    return in_maps


LAST_RESULT = None


def kernel(**inputs):
    x = np.asarray(inputs["x"], np.float32)
    pos_h = np.asarray(inputs["pos_h"], np.float32)
    pos_w = np.asarray(inputs["pos_w"], np.float32)
    weights = {
        "h": tuple(np.asarray(inputs[f"W{t}_h"], np.float32) for t in "qkvo"),
        "w": tuple(np.asarray(inputs[f"W{t}_w"], np.float32) for t in "qkvo"),
    }
    bosum = (
        np.asarray(inputs["bo_h"], np.float32) + np.asarray(inputs["bo_w"], np.float32)
    ).reshape(C, 1)

    in_maps = _host_prep(x, pos_h, pos_w, weights)
    for m in in_maps:
        m["bosum"] = bosum

    nc = _get_nc()
    trace_kw = {}
    if os.environ.get("AXIAL_TRACE", "0") == "1":
        trace_kw = dict(trace=True, trace_cores=[0], tmpdir="/tmp/axial_trace")
    res = run_bass_kernel_spmd(nc, in_maps, list(range(N_CORES)), **trace_kw)
    global LAST_RESULT
    LAST_RESULT = res

    out = np.empty((B, C, H, W), np.float32)
    for core in range(N_CORES):
        b, s = core // 2, core % 2
        o = res.results[core]["out"].reshape(C, H, WL)
        out[b, :, :, s * WL : (s + 1) * WL] = o
    return out


if __name__ == "__main__":
    import reference

    inputs = {k: np.asarray(v) for k, v in reference.setup_inputs().items()}
    got = kernel(**inputs)
    import jax

    with jax.default_device(jax.devices("cpu")[0]):
        exp = np.asarray(reference.reference(**reference.setup_inputs()))
    err = np.abs(got - exp).max() / np.abs(exp).max()
    print("rel err:", err)


# revision 19
# speedup vs baseline: 1.1766x; 1.1766x over previous
import sys

sys.path.insert(0, "/opt/trn_rl_repo")

import os

import numpy as np
import ml_dtypes

import concourse.bass as bass
import concourse.mybir as mybir
from concourse.tile import TileContext
from concourse.bass_utils import run_bass_kernel_spmd


def _split_multiwait_drains(nc):
    """This walrus build only encodes one sem-wait per instruction; hoist
    extra waits onto preceding same-engine NoOps (engines execute their
    instructions in block order, so the waits remain equivalent)."""
    import bass_rust

    uid = [0]
    for fn in nc.m.functions:
        for blk in fn.blocks:
            out, changed = [], False
            for inst in blk.instructions:
                si = getattr(inst, "sync_info", None)
                if si is not None and si.on_wait and len(si.on_wait) > 1:
                    waits = list(si.on_wait)
                    for w in waits[:-1]:
                        n = bass_rust.InstNoOp(name=f"syncw_{uid[0]}", ins=[], outs=[])
                        uid[0] += 1
                        n.engine = inst.engine
                        n.sync_info = bass_rust.SyncInfo(on_wait=[w], on_update=[])
                        out.append(n)
                    si.on_wait = [waits[-1]]
                    changed = True
                out.append(inst)
            if changed:
                blk.instructions = out


B, C, H, W = 4, 128, 128, 128
HEADS, DH = 8, 16
WL = 64  # per-core w-slice (2 cores per batch image)
N_CORES = 8
MB = 16  # sequences per megabatch

FP32 = mybir.dt.float32
BF16 = mybir.dt.bfloat16
BF16_NP = ml_dtypes.bfloat16

AXES = ("h", "w")


def _build_nc():
    nc = bass.Bass()

    x = nc.declare_dram_parameter("x", [C, H * W], FP32, isOutput=False)
    phw = nc.declare_dram_parameter("phw", [C, H * W], BF16, isOutput=False)
    sel = nc.declare_dram_parameter("sel", [C, C], BF16, isOutput=False)
    bosum = nc.declare_dram_parameter("bosum", [C, 1], FP32, isOutput=False)
    wts = {}
    for ax in AXES:
        for wn in ("wk", "woa", "wob") + tuple(f"wqm{h}" for h in range(8)):
            wts[f"{wn}_{ax}"] = nc.declare_dram_parameter(
                f"{wn}_{ax}", [C, C], BF16, isOutput=False
            )
        wts[f"wv_{ax}"] = nc.declare_dram_parameter(
            f"wv_{ax}", [C, 2 * C], BF16, isOutput=False
        )
    out = nc.declare_dram_parameter("out", [C, H * WL], FP32, isOutput=True)

    EXP = mybir.ActivationFunctionType.Exp
    LOG = mybir.ActivationFunctionType.Ln
    ADD = mybir.AluOpType.add
    MULT = mybir.AluOpType.mult

    QCH = 512  # psum chunk (one bank of fp32)

    with TileContext(nc) as tc:
        with (
            tc.tile_pool(name="big", bufs=1) as big,
            tc.tile_pool(name="wpool", bufs=1) as wpool,
            tc.tile_pool(name="slab", bufs=1) as slab_pool,
            tc.tile_pool(name="etsb", bufs=3) as etsb_pool,
            tc.tile_pool(name="xwp", bufs=2) as xw_pool,
            tc.tile_pool(name="misc", bufs=2) as misc_pool,
            tc.tile_pool(name="xin", bufs=2) as xin_pool,
            tc.tile_pool(name="scps", bufs=2, space="PSUM") as scps_pool,
            tc.tile_pool(name="ups", bufs=1, space="PSUM") as ups_pool,
            tc.tile_pool(name="pps", bufs=2, space="PSUM") as pps_pool,
        ):
            # ---- stage weights (DMAs issued from sync engine; it is idle) ----
            wsb = {}
            for k, t in wts.items():
                wt = wpool.tile(list(t.shape), t.dtype, tag=k)
                nc.sync.dma_start(out=wt[:], in_=t[:])
                wsb[k] = wt
            sel_sb = wpool.tile([C, C], BF16, tag="sel")
            nc.sync.dma_start(out=sel_sb[:], in_=sel[:])
            bos_sb = wpool.tile([C, 1], FP32, tag="bos")
            nc.sync.dma_start(out=bos_sb[:], in_=bosum[:])

            # ---- resident slabs ----
            xp_sb = big.tile([C, H * W], BF16, tag="xp")
            th_sb = big.tile([C, H * WL], BF16, tag="th")  # [c, h, wl]
            tw_sb = big.tile([C, H * WL], BF16, tag="tw")  # [c, h, wl]

            # xp = x + phw (x streamed through small rotating tiles)
            NCH = 8
            CH = (H * W) // NCH
            for k in range(NCH):
                cs = slice(k * CH, (k + 1) * CH)
                xt = xin_pool.tile([C, CH], FP32, tag="xin")
                nc.sync.dma_start(out=xt[:], in_=x[:, cs])
                nc.sync.dma_start(out=xp_sb[:, cs], in_=phw[:, cs])
                nc.vector.tensor_tensor(
                    out=xp_sb[:, cs], in0=xp_sb[:, cs], in1=xt[:], op=ADD
                )

            # v tiles with persistent ones columns
            v_tiles = []
            for j in range(3):
                vt = wpool.tile([C, 2 * C], BF16, tag=f"vsb{j}")
                vt3 = vt[:].rearrange("p (h c) -> p h c", c=32)
                nc.vector.memset(vt3[:, :, 16:], 1.0)
                v_tiles.append(vt)

            xp3 = xp_sb[:].rearrange("p (h w) -> p h w", w=W)

            def _copy_v(dst, src):
                nc.vector.tensor_copy(dst, src)

            def _copy_g(dst, src):
                nc.gpsimd.tensor_copy(dst, src)

            def _copy_s(dst, src):
                nc.scalar.copy(dst, src)

            # PSUM-reading copies: only DVE and Act may touch PSUM
            copy_fns = [_copy_v, _copy_s]
            eng_i = [0]

            def next_eng():
                e = copy_fns[eng_i[0] % 2]
                eng_i[0] += 1
                return e

            # shared slabs, sized for the larger (H) axis
            q_sl = slab_pool.tile([C, MB * 8 * H], BF16, tag="q")
            k_sl = slab_pool.tile([C, MB * W], BF16, tag="k")
            u_sl = slab_pool.tile([C, MB * 2 * H], BF16, tag="u")
            rn_sl = slab_pool.tile([C, MB * 2 * H], FP32, tag="rn")
            on_sl = slab_pool.tile([C, MB * 2 * H], BF16, tag="on")

            def run_axis(ax, nseq, nq, out_slab):
                """ax: 'h' (seq=w-col, nq=H) or 'w' (seq=h-row, nq=WL)."""
                n_mb = nseq // MB
                q4 = q_sl[:, : MB * 8 * nq].rearrange(
                    "p (s g q) -> p s g q", g=8, q=nq
                )

                for mb in range(n_mb):
                    # ---------- stage seq-major contiguous xp copy ----------
                    if ax == "h":
                        # xw cols (wl, h) <- xp[c, h, w-slice] transposed
                        xw_t = xw_pool.tile([C, MB * H], BF16, tag="xw")
                        xw = xw_t[:, : MB * nq]
                        nc.gpsimd.tensor_copy(
                            xw.rearrange("p (s q) -> p s q", q=nq),
                            xp3[:, :, mb * MB : (mb + 1) * MB].rearrange(
                                "p h w -> p w h"
                            ),
                        )
                        mk_flat = xw  # kv positions = same cols
                        m_kv = H
                    else:
                        # q cols (h-row, w<WL) contiguous-staged; kv rows direct
                        xw_t = xw_pool.tile([C, MB * H], BF16, tag="xw")
                        xw = xw_t[:, : MB * nq]
                        nc.gpsimd.tensor_copy(
                            xw.rearrange("p (s q) -> p s q", q=nq),
                            xp3[:, mb * MB : (mb + 1) * MB, 0:WL],
                        )
                        mk_flat = xp_sb[
                            :, mb * MB * W : (mb + 1) * MB * W
                        ]  # contiguous (h, w)
                        m_kv = W

                    # k projection first (no xw dependency; hides the DMA)
                    spk = QCH // m_kv
                    nkch = MB // spk
                    kd3 = k_sl[:, : MB * m_kv].rearrange("p (s q) -> p s q", q=m_kv)
                    for ck in range(nkch):
                        pt = pps_pool.tile([C, QCH], FP32, tag="pp")
                        nc.tensor.matmul(
                            pt[:],
                            wsb[f"wk_{ax}"][:],
                            mk_flat[:, ck * QCH : (ck + 1) * QCH],
                        )
                        next_eng()(
                            kd3[:, ck * spk : (ck + 1) * spk, :],
                            pt[:].rearrange("p (s q) -> p s q", q=m_kv),
                        )
                    # q projections: 8 masked weights; psum-chunked
                    spq = QCH // nq  # seqs per chunk
                    nqch = MB // spq
                    for h in range(8):
                        for ck in range(nqch):
                            pt = pps_pool.tile([C, QCH], FP32, tag="pp")
                            nc.tensor.matmul(
                                pt[:],
                                wsb[f"wqm{h}_{ax}"][:],
                                xw[:, ck * QCH : (ck + 1) * QCH],
                            )
                            next_eng()(
                                q4[:, ck * spq : (ck + 1) * spq, h, :],
                                pt[:].rearrange("p (s q) -> p s q", q=nq),
                            )

                    us4 = u_sl[:, : MB * 2 * nq].rearrange(
                        "p (v s q) -> p v s q", v=2, q=nq
                    )

                    # ---------- per-sequence attention ----------
                    for si in range(MB):
                        seq = mb * MB + si
                        if ax == "h":
                            rhs_kv = xw[:, si * H : (si + 1) * H]  # contiguous
                        else:
                            rhs_kv = xp3[:, seq, :]  # [C, W] contiguous

                        # v projection: stationary xp-seq, moving wv (aug)
                        v_ps = pps_pool.tile([C, QCH], FP32, tag="pp")
                        nc.tensor.matmul(
                            v_ps[:, : 2 * C], rhs_kv, wsb[f"wv_{ax}"][:]
                        )
                        v_sb = v_tiles[si % 3]
                        vap_src = v_ps[:, : 2 * C].rearrange(
                            "p (h c) -> p h c", c=32
                        )[:, :, :16]
                        vap_dst = v_sb[:].rearrange("p (h c) -> p h c", c=32)[
                            :, :, :16
                        ]
                        nc.vector.tensor_copy(vap_dst, vap_src)

                        # scores: masked q blocks vs natural K stationary
                        # (moving capped at 512 cols by the matmul ISA)
                        et_sb = etsb_pool.tile([C, 8 * nq], BF16, tag="et")
                        sc_ps = scps_pool.tile([C, 8 * nq], FP32, tag="sc")
                        n_sc = max(1, (8 * nq) // 512)
                        scw = (8 * nq) // n_sc
                        for sck in range(n_sc):
                            nc.tensor.matmul(
                                sc_ps[:, sck * scw : (sck + 1) * scw],
                                k_sl[:, si * m_kv : (si + 1) * m_kv],
                                q_sl[
                                    :,
                                    si * 8 * nq + sck * scw : si * 8 * nq
                                    + (sck + 1) * scw,
                                ],
                            )
                        nc.scalar.activation(et_sb[:], sc_ps[:], EXP)

                        # AV: one matmul per half (4 heads at once + sum rows)
                        u_ps = ups_pool.tile([C, 8 * nq], FP32, tag="u")
                        nc.tensor.matmul(
                            u_ps[:, 0 : 4 * nq], v_sb[:, 0:C], et_sb[:, 0 : 4 * nq]
                        )
                        nc.tensor.matmul(
                            u_ps[:, 4 * nq : 8 * nq],
                            v_sb[:, C : 2 * C],
                            et_sb[:, 4 * nq : 8 * nq],
                        )
                        # compaction: per 32-row group g, gather block g of both
                        # halves -> u_sl[32g:32g+32, (si, :, :)]
                        up4 = u_ps[:].rearrange(
                            "p (v g q) -> p v g q", v=2, q=nq
                        )
                        for g in range(4):
                            next_eng()(
                                us4[32 * g : 32 * g + 32, :, si, :],
                                up4[32 * g : 32 * g + 32, :, g, :],
                            )

                    # ---------- batched normalize + output projection ----------
                    u_tot = MB * 2 * nq
                    nsch = u_tot // QCH
                    for ck in range(nsch):
                        cs = slice(ck * QCH, (ck + 1) * QCH)
                        rb_ps = pps_pool.tile([C, QCH], FP32, tag="pp")
                        nc.tensor.matmul(rb_ps[:], sel_sb[:], u_sl[:, cs])
                        ls_t = misc_pool.tile([C, QCH], FP32, tag="ls")
                        nc.scalar.activation(ls_t[:], rb_ps[:], LOG)
                        nc.scalar.activation(rn_sl[:, cs], ls_t[:], EXP, scale=-1.0)
                    for ck in range(2):
                        cs = slice(ck * u_tot // 2, (ck + 1) * u_tot // 2)
                        nc.vector.tensor_tensor(
                            out=on_sl[:, cs], in0=u_sl[:, cs], in1=rn_sl[:, cs],
                            op=MULT,
                        )

                    spy = QCH // nq
                    nych = MB // spy
                    for ck in range(nych):
                        y_ps = pps_pool.tile([C, QCH], FP32, tag="pp")
                        nc.tensor.matmul(
                            y_ps[:],
                            wsb[f"woa_{ax}"][:],
                            on_sl[:, ck * QCH : (ck + 1) * QCH],
                            start=True, stop=False,
                        )
                        nc.tensor.matmul(
                            y_ps[:],
                            wsb[f"wob_{ax}"][:],
                            on_sl[:, u_tot // 2 + ck * QCH : u_tot // 2 + (ck + 1) * QCH],
                            start=False, stop=True,
                        )
                        if ax == "h":
                            # y cols (wl, h) -> th[c, h, wl]
                            dst = out_slab[:].rearrange("p (h w) -> p h w", w=WL)[
                                :, :,
                                mb * MB + ck * spy : mb * MB + (ck + 1) * spy,
                            ].rearrange("p h w -> p w h")
                            src = y_ps[:].rearrange("p (s q) -> p s q", q=nq)
                        else:
                            # y cols (h, w) -> tw[c, h, w] contiguous
                            dst = out_slab[
                                :,
                                (mb * MB + ck * spy) * nq : (mb * MB + (ck + 1) * spy) * nq,
                            ]
                            src = y_ps[:]
                        next_eng()(dst, src)

            run_axis("h", WL, H, th_sb)
            run_axis("w", H, WL, tw_sb)

            # ============ final: out = th + tw + x(local) + bias ============
            x3d = x[:].rearrange("p (h w) -> p h w", w=W)
            NFC = 8
            FH = (H * WL) // NFC  # flat cols per chunk
            HR = H // NFC  # h-rows per chunk
            for k in range(NFC):
                cs = slice(k * FH, (k + 1) * FH)
                xt = xin_pool.tile([C, FH], FP32, tag="xr")
                nc.sync.dma_start(
                    out=xt[:].rearrange("p (h w) -> p h w", w=WL),
                    in_=x3d[:, k * HR : (k + 1) * HR, 0:WL],
                )
                t1 = misc_pool.tile([C, FH], BF16, tag="f1")
                nc.vector.tensor_tensor(
                    out=t1[:], in0=th_sb[:, cs], in1=tw_sb[:, cs], op=ADD
                )
                nc.vector.tensor_scalar_add(out=t1[:], in0=t1[:], scalar1=bos_sb[:])
                ot = xin_pool.tile([C, FH], FP32, tag="ot")
                nc.vector.tensor_tensor(out=ot[:], in0=t1[:], in1=xt[:], op=ADD)
                nc.sync.dma_start(out=out[:, cs], in_=ot[:])

    _split_multiwait_drains(nc)
    return nc


_NC_CACHE = None


def _get_nc():
    global _NC_CACHE
    if _NC_CACHE is None:
        _NC_CACHE = _build_nc()
    return _NC_CACHE


def _host_prep(x, pos_h, pos_w, weights):
    """build per-core input maps"""
    scale = DH ** -0.5
    phw = (pos_h + pos_w)[0]  # [C, H, W]

    def grouped_rows(Wm, heads_sel):
        out = np.zeros((C, C), np.float32)
        for g, h in enumerate(heads_sel):
            out[32 * g : 32 * g + 16, :] = Wm[16 * h : 16 * h + 16, :]
        return out

    def v_aug(Wm):
        out = np.zeros((C, 2 * C), np.float32)
        for h in range(HEADS):
            out[:, 32 * h : 32 * h + 16] = Wm[:, 16 * h : 16 * h + 16]
        return out

    sel = np.zeros((C, C), np.float32)
    for q in range(C):
        sel[32 * (q // 32) + 16, q] = 1.0

    base = {"sel": sel.astype(BF16_NP)}
    for ax in AXES:
        Wq, Wk, Wv, Wo = weights[ax]
        Wqs = Wq * scale
        for h in range(HEADS):
            wm = np.zeros((C, C), np.float32)
            wm[:, 16 * h : 16 * h + 16] = Wqs[:, 16 * h : 16 * h + 16]
            base[f"wqm{h}_{ax}"] = wm.astype(BF16_NP)
        base[f"wk_{ax}"] = Wk.astype(BF16_NP)
        base[f"wv_{ax}"] = v_aug(Wv).astype(BF16_NP)
        base[f"woa_{ax}"] = grouped_rows(Wo, [0, 1, 2, 3]).astype(BF16_NP)
        base[f"wob_{ax}"] = grouped_rows(Wo, [4, 5, 6, 7]).astype(BF16_NP)

    in_maps = []
    for core in range(N_CORES):
        b, s = core // 2, core % 2
        xb = x[b]
        pb = phw
        if s == 1:
            xb = np.concatenate([xb[:, :, WL:], xb[:, :, :WL]], axis=2)
            pb = np.concatenate([pb[:, :, WL:], pb[:, :, :WL]], axis=2)
        m = dict(base)
        m["x"] = np.ascontiguousarray(xb.reshape(C, H * W), np.float32)
        m["phw"] = np.ascontiguousarray(pb.reshape(C, H * W)).astype(BF16_NP)
        in_maps.append(m)
    return in_maps


LAST_RESULT = None


def kernel(**inputs):
    x = np.asarray(inputs["x"], np.float32)
    pos_h = np.asarray(inputs["pos_h"], np.float32)
    pos_w = np.asarray(inputs["pos_w"], np.float32)
    weights = {
        "h": tuple(np.asarray(inputs[f"W{t}_h"], np.float32) for t in "qkvo"),
        "w": tuple(np.asarray(inputs[f"W{t}_w"], np.float32) for t in "qkvo"),
    }
    bosum = (
        np.asarray(inputs["bo_h"], np.float32) + np.asarray(inputs["bo_w"], np.float32)
    ).reshape(C, 1)

    in_maps = _host_prep(x, pos_h, pos_w, weights)
    for m in in_maps:
        m["bosum"] = bosum

    nc = _get_nc()
    trace_kw = {}
    if os.environ.get("AXIAL_TRACE", "0") == "1":
        trace_kw = dict(trace=True, trace_cores=[0], tmpdir="/tmp/axial_trace")
    res = run_bass_kernel_spmd(nc, in_maps, list(range(N_CORES)), **trace_kw)
    global LAST_RESULT
    LAST_RESULT = res

    out = np.empty((B, C, H, W), np.float32)
    for core in range(N_CORES):
        b, s = core // 2, core % 2
        o = res.results[core]["out"].reshape(C, H, WL)
        out[b, :, :, s * WL : (s + 1) * WL] = o
    return out


if __name__ == "__main__":
    import reference

    inputs = {k: np.asarray(v) for k, v in reference.setup_inputs().items()}
    got = kernel(**inputs)
    import jax

    with jax.default_device(jax.devices("cpu")[0]):
        exp = np.asarray(reference.reference(**reference.setup_inputs()))
    err = np.abs(got - exp).max() / np.abs(exp).max()
    print("rel err:", err)


# revision 20
# speedup vs baseline: 1.2630x; 1.0734x over previous
import sys

sys.path.insert(0, "/opt/trn_rl_repo")

import os

import numpy as np
import ml_dtypes

import concourse.bass as bass
import concourse.mybir as mybir
from concourse.tile import TileContext
from concourse.bass_utils import run_bass_kernel_spmd


def _split_multiwait_drains(nc):
    """This walrus build only encodes one sem-wait per instruction; hoist
    extra waits onto preceding same-engine NoOps (engines execute their
    instructions in block order, so the waits remain equivalent)."""
    import bass_rust

    uid = [0]
    for fn in nc.m.functions:
        for blk in fn.blocks:
            out, changed = [], False
            for inst in blk.instructions:
                si = getattr(inst, "sync_info", None)
                if si is not None and si.on_wait and len(si.on_wait) > 1:
                    waits = list(si.on_wait)
                    for w in waits[:-1]:
                        n = bass_rust.InstNoOp(name=f"syncw_{uid[0]}", ins=[], outs=[])
                        uid[0] += 1
                        n.engine = inst.engine
                        n.sync_info = bass_rust.SyncInfo(on_wait=[w], on_update=[])
                        out.append(n)
                    si.on_wait = [waits[-1]]
                    changed = True
                out.append(inst)
            if changed:
                blk.instructions = out


B, C, H, W = 4, 128, 128, 128
HEADS, DH = 8, 16
WL = 64  # per-core w-slice (2 cores per batch image)
N_CORES = 8
MB = 16  # sequences per megabatch

FP32 = mybir.dt.float32
BF16 = mybir.dt.bfloat16
BF16_NP = ml_dtypes.bfloat16

AXES = ("h", "w")


def _build_nc():
    nc = bass.Bass()

    x = nc.declare_dram_parameter("x", [C, H * W], FP32, isOutput=False)
    phw = nc.declare_dram_parameter("phw", [C, H * W], BF16, isOutput=False)
    sel = nc.declare_dram_parameter("sel", [C, C], BF16, isOutput=False)
    bosum = nc.declare_dram_parameter("bosum", [C, 1], FP32, isOutput=False)
    wts = {}
    for ax in AXES:
        for wn in ("wk", "woa", "wob") + tuple(f"wqm{h}" for h in range(8)):
            wts[f"{wn}_{ax}"] = nc.declare_dram_parameter(
                f"{wn}_{ax}", [C, C], BF16, isOutput=False
            )
        wts[f"wv_{ax}"] = nc.declare_dram_parameter(
            f"wv_{ax}", [C, 2 * C], BF16, isOutput=False
        )
    out = nc.declare_dram_parameter("out", [C, H * WL], FP32, isOutput=True)

    EXP = mybir.ActivationFunctionType.Exp
    LOG = mybir.ActivationFunctionType.Ln
    ADD = mybir.AluOpType.add
    MULT = mybir.AluOpType.mult

    QCH = 512  # psum chunk (one bank of fp32)

    with TileContext(nc) as tc:
        with (
            tc.tile_pool(name="big", bufs=1) as big,
            tc.tile_pool(name="wpool", bufs=1) as wpool,
            tc.tile_pool(name="slab", bufs=1) as slab_pool,
            tc.tile_pool(name="etsb", bufs=3) as etsb_pool,
            tc.tile_pool(name="xwp", bufs=2) as xw_pool,
            tc.tile_pool(name="misc", bufs=2) as misc_pool,
            tc.tile_pool(name="xin", bufs=2) as xin_pool,
            tc.tile_pool(name="scps", bufs=2, space="PSUM") as scps_pool,
            tc.tile_pool(name="ups", bufs=1, space="PSUM") as ups_pool,
            tc.tile_pool(name="pps", bufs=2, space="PSUM") as pps_pool,
        ):
            # ---- stage weights (DMAs issued from sync engine; it is idle) ----
            wsb = {}
            for k, t in wts.items():
                wt = wpool.tile(list(t.shape), t.dtype, tag=k)
                nc.sync.dma_start(out=wt[:], in_=t[:])
                wsb[k] = wt
            sel_sb = wpool.tile([C, C], BF16, tag="sel")
            nc.sync.dma_start(out=sel_sb[:], in_=sel[:])
            bos_sb = wpool.tile([C, 1], FP32, tag="bos")
            nc.sync.dma_start(out=bos_sb[:], in_=bosum[:])

            # ---- resident slabs ----
            xp_sb = big.tile([C, H * W], BF16, tag="xp")
            th_sb = big.tile([C, H * WL], BF16, tag="th")  # [c, h, wl]
            tw_sb = big.tile([C, H * WL], BF16, tag="tw")  # [c, h, wl]

            # xp = x + phw (x streamed through small rotating tiles)
            NCH = 8
            CH = (H * W) // NCH
            for k in range(NCH):
                cs = slice(k * CH, (k + 1) * CH)
                xt = xin_pool.tile([C, CH], FP32, tag="xin")
                nc.sync.dma_start(out=xt[:], in_=x[:, cs])
                nc.sync.dma_start(out=xp_sb[:, cs], in_=phw[:, cs])
                nc.vector.tensor_tensor(
                    out=xp_sb[:, cs], in0=xp_sb[:, cs], in1=xt[:], op=ADD
                )

            # v tiles with persistent ones columns
            v_tiles = []
            for j in range(3):
                vt = wpool.tile([C, 2 * C], BF16, tag=f"vsb{j}")
                vt3 = vt[:].rearrange("p (h c) -> p h c", c=32)
                nc.vector.memset(vt3[:, :, 16:], 1.0)
                v_tiles.append(vt)

            xp3 = xp_sb[:].rearrange("p (h w) -> p h w", w=W)

            def _copy_v(dst, src):
                nc.vector.tensor_copy(dst, src)

            def _copy_g(dst, src):
                nc.gpsimd.tensor_copy(dst, src)

            def _copy_s(dst, src):
                nc.scalar.copy(dst, src)

            # PSUM-reading copies: only DVE and Act may touch PSUM
            copy_fns = [_copy_v, _copy_s]
            eng_i = [0]

            def next_eng():
                e = copy_fns[eng_i[0] % 2]
                eng_i[0] += 1
                return e

            # shared slabs, sized for the larger (H) axis
            q_sl = slab_pool.tile([C, MB * 8 * H], BF16, tag="q")
            k_sl = slab_pool.tile([C, MB * W], BF16, tag="k")
            u_sl = slab_pool.tile([C, MB * 2 * H], BF16, tag="u")
            rn_sl = slab_pool.tile([C, MB * 2 * H], FP32, tag="rn")
            on_sl = slab_pool.tile([C, MB * 2 * H], BF16, tag="on")

            def run_axis(ax, nseq, nq, out_slab):
                """ax: 'h' (seq=w-col, nq=H) or 'w' (seq=h-row, nq=WL)."""
                n_mb = nseq // MB
                q4 = q_sl[:, : MB * 8 * nq].rearrange(
                    "p (s g q) -> p s g q", g=8, q=nq
                )

                for mb in range(n_mb):
                    # ---------- stage seq-major contiguous xp copy ----------
                    if ax == "h":
                        # xw cols (wl, h) <- xp[c, h, w-slice] transposed
                        xw_t = xw_pool.tile([C, MB * H], BF16, tag="xw")
                        xw = xw_t[:, : MB * nq]
                        nc.gpsimd.tensor_copy(
                            xw.rearrange("p (s q) -> p s q", q=nq),
                            xp3[:, :, mb * MB : (mb + 1) * MB].rearrange(
                                "p h w -> p w h"
                            ),
                        )
                        mk_flat = xw  # kv positions = same cols
                        m_kv = H
                    else:
                        # q cols (h-row, w<WL) contiguous-staged; kv rows direct
                        xw_t = xw_pool.tile([C, MB * H], BF16, tag="xw")
                        xw = xw_t[:, : MB * nq]
                        nc.gpsimd.tensor_copy(
                            xw.rearrange("p (s q) -> p s q", q=nq),
                            xp3[:, mb * MB : (mb + 1) * MB, 0:WL],
                        )
                        mk_flat = xp_sb[
                            :, mb * MB * W : (mb + 1) * MB * W
                        ]  # contiguous (h, w)
                        m_kv = W

                    # k projection first (no xw dependency; hides the DMA)
                    spk = QCH // m_kv
                    nkch = MB // spk
                    kd3 = k_sl[:, : MB * m_kv].rearrange("p (s q) -> p s q", q=m_kv)
                    for ck in range(nkch):
                        pt = pps_pool.tile([C, QCH], FP32, tag="pp")
                        nc.tensor.matmul(
                            pt[:],
                            wsb[f"wk_{ax}"][:],
                            mk_flat[:, ck * QCH : (ck + 1) * QCH],
                        )
                        next_eng()(
                            kd3[:, ck * spk : (ck + 1) * spk, :],
                            pt[:].rearrange("p (s q) -> p s q", q=m_kv),
                        )
                    # q projections: 8 masked weights; psum-chunked
                    spq = QCH // nq  # seqs per chunk
                    nqch = MB // spq
                    for h in range(8):
                        for ck in range(nqch):
                            pt = pps_pool.tile([C, QCH], FP32, tag="pp")
                            nc.tensor.matmul(
                                pt[:],
                                wsb[f"wqm{h}_{ax}"][:],
                                xw[:, ck * QCH : (ck + 1) * QCH],
                            )
                            next_eng()(
                                q4[:, ck * spq : (ck + 1) * spq, h, :],
                                pt[:].rearrange("p (s q) -> p s q", q=nq),
                            )

                    us4 = u_sl[:, : MB * 2 * nq].rearrange(
                        "p (v s q) -> p v s q", v=2, q=nq
                    )

                    # ---------- per-sequence attention (sw-pipelined) ----------
                    # stage A(s): v-proj + scores + exp for seq s
                    # stage B(s): AV + compaction for seq s (lags one seq so
                    # the tensor engine never blocks on exp(s))
                    et_tiles = [None] * MB

                    def stage_a(si):
                        if ax == "h":
                            rhs_kv = xw[:, si * H : (si + 1) * H]
                        else:
                            rhs_kv = xp3[:, mb * MB + si, :]
                        v_ps = pps_pool.tile([C, QCH], FP32, tag="pp")
                        nc.tensor.matmul(
                            v_ps[:, : 2 * C], rhs_kv, wsb[f"wv_{ax}"][:]
                        )
                        v_sb = v_tiles[si % 3]
                        vap_src = v_ps[:, : 2 * C].rearrange(
                            "p (h c) -> p h c", c=32
                        )[:, :, :16]
                        vap_dst = v_sb[:].rearrange("p (h c) -> p h c", c=32)[
                            :, :, :16
                        ]
                        nc.vector.tensor_copy(vap_dst, vap_src)

                        et_sb = etsb_pool.tile([C, 8 * nq], BF16, tag="et")
                        et_tiles[si] = et_sb
                        sc_ps = scps_pool.tile([C, 8 * nq], FP32, tag="sc")
                        n_sc = max(1, (8 * nq) // 512)
                        scw = (8 * nq) // n_sc
                        for sck in range(n_sc):
                            nc.tensor.matmul(
                                sc_ps[:, sck * scw : (sck + 1) * scw],
                                k_sl[:, si * m_kv : (si + 1) * m_kv],
                                q_sl[
                                    :,
                                    si * 8 * nq + sck * scw : si * 8 * nq
                                    + (sck + 1) * scw,
                                ],
                            )
                        nc.scalar.activation(et_sb[:], sc_ps[:], EXP)

                    def stage_b(si):
                        et_sb = et_tiles[si]
                        v_sb = v_tiles[si % 3]
                        u_ps = ups_pool.tile([C, 8 * nq], FP32, tag="u")
                        nc.tensor.matmul(
                            u_ps[:, 0 : 4 * nq], v_sb[:, 0:C], et_sb[:, 0 : 4 * nq]
                        )
                        nc.tensor.matmul(
                            u_ps[:, 4 * nq : 8 * nq],
                            v_sb[:, C : 2 * C],
                            et_sb[:, 4 * nq : 8 * nq],
                        )
                        up4 = u_ps[:].rearrange("p (v g q) -> p v g q", v=2, q=nq)
                        for g in range(4):
                            cp = nc.scalar.copy if g == 3 else nc.vector.tensor_copy
                            cp(
                                us4[32 * g : 32 * g + 32, :, si, :],
                                up4[32 * g : 32 * g + 32, :, g, :],
                            )

                    stage_a(0)
                    for si in range(MB):
                        if si + 1 < MB:
                            stage_a(si + 1)
                        stage_b(si)

                    # ---------- batched normalize + output projection ----------
                    u_tot = MB * 2 * nq
                    nsch = u_tot // QCH
                    for ck in range(nsch):
                        cs = slice(ck * QCH, (ck + 1) * QCH)
                        rb_ps = pps_pool.tile([C, QCH], FP32, tag="pp")
                        nc.tensor.matmul(rb_ps[:], sel_sb[:], u_sl[:, cs])
                        ls_t = misc_pool.tile([C, QCH], FP32, tag="ls")
                        nc.scalar.activation(ls_t[:], rb_ps[:], LOG)
                        nc.scalar.activation(rn_sl[:, cs], ls_t[:], EXP, scale=-1.0)
                    for ck in range(2):
                        cs = slice(ck * u_tot // 2, (ck + 1) * u_tot // 2)
                        nc.vector.tensor_tensor(
                            out=on_sl[:, cs], in0=u_sl[:, cs], in1=rn_sl[:, cs],
                            op=MULT,
                        )

                    spy = QCH // nq
                    nych = MB // spy
                    for ck in range(nych):
                        y_ps = pps_pool.tile([C, QCH], FP32, tag="pp")
                        nc.tensor.matmul(
                            y_ps[:],
                            wsb[f"woa_{ax}"][:],
                            on_sl[:, ck * QCH : (ck + 1) * QCH],
                            start=True, stop=False,
                        )
                        nc.tensor.matmul(
                            y_ps[:],
                            wsb[f"wob_{ax}"][:],
                            on_sl[:, u_tot // 2 + ck * QCH : u_tot // 2 + (ck + 1) * QCH],
                            start=False, stop=True,
                        )
                        if ax == "h":
                            # y cols (wl, h) -> th[c, h, wl]
                            dst = out_slab[:].rearrange("p (h w) -> p h w", w=WL)[
                                :, :,
                                mb * MB + ck * spy : mb * MB + (ck + 1) * spy,
                            ].rearrange("p h w -> p w h")
                            src = y_ps[:].rearrange("p (s q) -> p s q", q=nq)
                        else:
                            # y cols (h, w) -> tw[c, h, w] contiguous
                            dst = out_slab[
                                :,
                                (mb * MB + ck * spy) * nq : (mb * MB + (ck + 1) * spy) * nq,
                            ]
                            src = y_ps[:]
                        next_eng()(dst, src)

            run_axis("h", WL, H, th_sb)
            run_axis("w", H, WL, tw_sb)

            # ============ final: out = th + tw + x(local) + bias ============
            x3d = x[:].rearrange("p (h w) -> p h w", w=W)
            NFC = 8
            FH = (H * WL) // NFC  # flat cols per chunk
            HR = H // NFC  # h-rows per chunk
            for k in range(NFC):
                cs = slice(k * FH, (k + 1) * FH)
                xt = xin_pool.tile([C, FH], FP32, tag="xr")
                nc.sync.dma_start(
                    out=xt[:].rearrange("p (h w) -> p h w", w=WL),
                    in_=x3d[:, k * HR : (k + 1) * HR, 0:WL],
                )
                t1 = misc_pool.tile([C, FH], BF16, tag="f1")
                nc.vector.tensor_tensor(
                    out=t1[:], in0=th_sb[:, cs], in1=tw_sb[:, cs], op=ADD
                )
                nc.vector.tensor_scalar_add(out=t1[:], in0=t1[:], scalar1=bos_sb[:])
                ot = xin_pool.tile([C, FH], FP32, tag="ot")
                nc.vector.tensor_tensor(out=ot[:], in0=t1[:], in1=xt[:], op=ADD)
                nc.sync.dma_start(out=out[:, cs], in_=ot[:])

    _split_multiwait_drains(nc)
    return nc


_NC_CACHE = None


def _get_nc():
    global _NC_CACHE
    if _NC_CACHE is None:
        _NC_CACHE = _build_nc()
    return _NC_CACHE


def _host_prep(x, pos_h, pos_w, weights):
    """build per-core input maps"""
    scale = DH ** -0.5
    phw = (pos_h + pos_w)[0]  # [C, H, W]

    def grouped_rows(Wm, heads_sel):
        out = np.zeros((C, C), np.float32)
        for g, h in enumerate(heads_sel):
            out[32 * g : 32 * g + 16, :] = Wm[16 * h : 16 * h + 16, :]
        return out

    def v_aug(Wm):
        out = np.zeros((C, 2 * C), np.float32)
        for h in range(HEADS):
            out[:, 32 * h : 32 * h + 16] = Wm[:, 16 * h : 16 * h + 16]
        return out

    sel = np.zeros((C, C), np.float32)
    for q in range(C):
        sel[32 * (q // 32) + 16, q] = 1.0

    base = {"sel": sel.astype(BF16_NP)}
    for ax in AXES:
        Wq, Wk, Wv, Wo = weights[ax]
        Wqs = Wq * scale
        for h in range(HEADS):
            wm = np.zeros((C, C), np.float32)
            wm[:, 16 * h : 16 * h + 16] = Wqs[:, 16 * h : 16 * h + 16]
            base[f"wqm{h}_{ax}"] = wm.astype(BF16_NP)
        base[f"wk_{ax}"] = Wk.astype(BF16_NP)
        base[f"wv_{ax}"] = v_aug(Wv).astype(BF16_NP)
        base[f"woa_{ax}"] = grouped_rows(Wo, [0, 1, 2, 3]).astype(BF16_NP)
        base[f"wob_{ax}"] = grouped_rows(Wo, [4, 5, 6, 7]).astype(BF16_NP)

    in_maps = []
    for core in range(N_CORES):
        b, s = core // 2, core % 2
        xb = x[b]
        pb = phw
        if s == 1:
            xb = np.concatenate([xb[:, :, WL:], xb[:, :, :WL]], axis=2)
            pb = np.concatenate([pb[:, :, WL:], pb[:, :, :WL]], axis=2)
        m = dict(base)
        m["x"] = np.ascontiguousarray(xb.reshape(C, H * W), np.float32)
        m["phw"] = np.ascontiguousarray(pb.reshape(C, H * W)).astype(BF16_NP)
        in_maps.append(m)
    return in_maps


LAST_RESULT = None


def kernel(**inputs):
    x = np.asarray(inputs["x"], np.float32)
    pos_h = np.asarray(inputs["pos_h"], np.float32)
    pos_w = np.asarray(inputs["pos_w"], np.float32)
    weights = {
        "h": tuple(np.asarray(inputs[f"W{t}_h"], np.float32) for t in "qkvo"),
        "w": tuple(np.asarray(inputs[f"W{t}_w"], np.float32) for t in "qkvo"),
    }
    bosum = (
        np.asarray(inputs["bo_h"], np.float32) + np.asarray(inputs["bo_w"], np.float32)
    ).reshape(C, 1)

    in_maps = _host_prep(x, pos_h, pos_w, weights)
    for m in in_maps:
        m["bosum"] = bosum

    nc = _get_nc()
    trace_kw = {}
    if os.environ.get("AXIAL_TRACE", "0") == "1":
        trace_kw = dict(trace=True, trace_cores=[0], tmpdir="/tmp/axial_trace")
    res = run_bass_kernel_spmd(nc, in_maps, list(range(N_CORES)), **trace_kw)
    global LAST_RESULT
    LAST_RESULT = res

    out = np.empty((B, C, H, W), np.float32)
    for core in range(N_CORES):
        b, s = core // 2, core % 2
        o = res.results[core]["out"].reshape(C, H, WL)
        out[b, :, :, s * WL : (s + 1) * WL] = o
    return out


if __name__ == "__main__":
    import reference

    inputs = {k: np.asarray(v) for k, v in reference.setup_inputs().items()}
    got = kernel(**inputs)
    import jax

    with jax.default_device(jax.devices("cpu")[0]):
        exp = np.asarray(reference.reference(**reference.setup_inputs()))
    err = np.abs(got - exp).max() / np.abs(exp).max()
    print("rel err:", err)
